# revision 20
# baseline (speedup 1.0000x reference)
"""Trainium2 Bass kernel for LLMAttention (B=2, T=2048, D=2048, H=16, HD=128).

Sharding: 8 cores = data parallel on B (2) x tensor parallel on heads (4 groups
of 4 heads).  Each core computes QKV projections for its 4 heads, per-head
QK RMSNorm + interleaved RoPE, causal attention, and a partial output
projection against its columns of Wo.  The host sums the 4 partials per batch.

v2 design notes (all hardcoded for the shapes above):
  - QKV projections run as fp8e4 DoubleRow matmuls (0.5 cycles/row) with a
    3-term residual correction accumulated in one PSUM group:
        q ~= x8@W8 + (xl*32)@(W/32) + (x/32)@(Wl*32)
    where x8=fp8(x), xl=x-x8, W8=fp8(64*W), Wl=64*W-W8.  All six operand
    variants are pre-quantized host-side so no scales are needed on device.
  - hd dimension of Q/K is host-permuted to [evens | odds] so RoPE pairs are
    contiguous 64-wide halves.  RoPE reads the QKV PSUM directly (no copy).
  - RMSNorm stats (sum of squares) are taken pre-RoPE from PSUM (rotations
    preserve norms).  Q's 1/rms is applied per-head via tensor_scalar on the
    rotated vectors; K's 1/rms (and the 1/sqrt(HD) score scale) rides in the
    exp()'s per-partition scale operand.
  - All transposes (qT, kT, ctxT) use DMA XBAR transposes instead of PE
    matmuls, freeing the tensor engine and the PSUM drain copies.
  - Softmax denominators come from a ones-column appended to V; the division
    rides in the cn PSUM->SBUF copy (tensor_scalar by the reciprocal).
  - Phase emission is interleaved (QKV proj / attention / output projection)
    to keep the PE continuously busy (p-state ramp) and overlap the exp()
    activation work with projection matmuls.
  - Output partials are written as bf16 (halves DMA); host sums in f32.
"""

import math
import os
from contextlib import ExitStack

import numpy as np
import ml_dtypes

import concourse.bass as bass
import concourse.bacc as bacc
import concourse.tile as tile
import concourse.mybir as mybir
from concourse.bass_utils import run_bass_kernel_spmd

B, T, D = 2, 2048, 2048
H, HD = 16, 128
ROPE_BASE = 10000.0
EPS = 1e-6

P = 128
TI = T // P            # 16 t-tiles of 128
DC = D // P            # 16 d-chunks of 128
HPC = 4                # heads per core
OC = HPC * HD          # 512 output cols per core
VW = HD + 1            # V width with ones column (129)
N_CORES = 8
SW = 64.0              # weight pre-scale for fp8 quantization
RS = 32.0              # residual pre-scale

BF16 = mybir.dt.bfloat16
F32 = mybir.dt.float32
FP8 = mybir.dt.float8e4
AF = mybir.ActivationFunctionType
ALU = mybir.AluOpType
DR = mybir.MatmulPerfMode.DoubleRow

_NC_CACHE = {}

W_NAMES = ("q", "k", "v")
TERMS = ("m", "r", "s")   # main, x-residual, w-residual


def _build_nc():
    nc = bacc.Bacc(
        "TRN2",
        target_bir_lowering=False,
        debug=False,
        enable_asserts=False,
        num_devices=N_CORES,
    )
    # x variants: [dp, ti, dc, tp] so one DMA per i-tile is a contiguous
    # 2KB-per-partition descriptor.
    xin = {
        t: nc.dram_tensor(f"x{t}", [P, TI, DC, P], FP8, kind="ExternalInput").ap()
        for t in TERMS
    }
    win = {
        (w, t): nc.dram_tensor(f"w{t}{w}", [P, DC, OC], FP8, kind="ExternalInput").ap()
        for w in W_NAMES
        for t in TERMS
    }
    wot = nc.dram_tensor("wot", [P, HPC, D], BF16, kind="ExternalInput").ap()
    cosf = nc.dram_tensor("cosf", [P, TI, HD], BF16, kind="ExternalInput").ap()
    sinf = nc.dram_tensor("sinf", [P, TI, HD], BF16, kind="ExternalInput").ap()
    maskd = nc.dram_tensor("maskd", [P, P], BF16, kind="ExternalInput").ap()
    out = nc.dram_tensor("out", [T, D], BF16, kind="ExternalOutput").ap()
    dbg = None
    if os.environ.get("KERNEL_DEBUG"):
        dbg = {
            nm: nc.dram_tensor(f"dbg_{nm}", [P, TI, HPC, w], F32, kind="ExternalOutput").ap()
            for nm, w in (("qT", P), ("kT", P), ("cT", P), ("v", VW))
        }
        dbg["recq"] = nc.dram_tensor("dbg_recq", [P, TI, HPC], F32, kind="ExternalOutput").ap()
        dbg["reck"] = nc.dram_tensor("dbg_reck", [P, TI, HPC], F32, kind="ExternalOutput").ap()

    with tile.TileContext(nc) as tc:
        _kernel_body(tc, xin, win, wot, cosf, sinf, maskd, out, dbg)

    nc.compile()
    return nc


def _kernel_body(tc, xin, win, wot, cosf, sinf, maskd, out, dbg=None):
    nc = tc.nc
    with ExitStack() as ctx:
        persist = ctx.enter_context(tc.tile_pool(name="persist", bufs=1))

        # ---- persistent SBUF ----
        w_sb = {}
        CH = 4  # weight DMA split granularity (d-chunks per DMA)
        for w in W_NAMES:
            for t in TERMS:
                w_sb[(w, t)] = persist.tile(
                    [P, DC, OC], FP8, tag=f"w{t}{w}", name=f"w{t}{w}"
                )
        cos_sb = persist.tile([P, TI, HD], BF16, tag="cos")
        sin_sb = persist.tile([P, TI, HD], BF16, tag="sin")
        mask_sb = persist.tile([P, P], BF16, tag="mask")
        wot_sb = persist.tile([P, HPC, D], BF16, tag="wot")

        qT = persist.tile([P, TI, HPC, P], BF16, tag="qT")
        kT = persist.tile([P, TI, HPC, P], BF16, tag="kT")
        cT = persist.tile([P, TI, HPC, P], BF16, tag="cT")
        v_sb = persist.tile([P, TI, HPC, VW], BF16, tag="v")
        recq = persist.tile([P, TI, HPC], F32, tag="recq")
        reck = persist.tile([P, TI, HPC], F32, tag="reck")
        eps_q = persist.tile([P, 1], F32, tag="eps_q")
        eps_k = persist.tile([P, 1], F32, tag="eps_k")

        # ---- weight/table DMAs, chunked so early matmuls start sooner ----
        # order: all of q's three variants first, then k, then v.
        for w in W_NAMES:
            for dq in range(0, DC, CH):
                for t in TERMS:
                    nc.sync.dma_start(
                        w_sb[(w, t)][:, dq : dq + CH, :],
                        win[(w, t)][:, dq : dq + CH, :],
                    )
        nc.sync.dma_start(cos_sb[:], cosf)
        nc.sync.dma_start(sin_sb[:], sinf)
        nc.sync.dma_start(mask_sb[:], maskd)
        nc.sync.dma_start(wot_sb[:], wot)
        nc.gpsimd.memset(v_sb[:, :, :, HD:VW], 1.0)
        nc.vector.memset(eps_q[:], SW * SW * EPS)
        nc.vector.memset(eps_k[:], SW * SW * HD * EPS)

        xpool = ctx.enter_context(tc.tile_pool(name="xp", bufs=2))
        qkvps = ctx.enter_context(tc.tile_pool(name="qkvps", bufs=2, space="PSUM"))
        sps = ctx.enter_context(tc.tile_pool(name="sps", bufs=2, space="PSUM"))
        cxps = ctx.enter_context(tc.tile_pool(name="cxps", bufs=1, space="PSUM"))
        work = ctx.enter_context(tc.tile_pool(name="work", bufs=2))
        small = ctx.enter_context(tc.tile_pool(name="small", bufs=4))
        pep = ctx.enter_context(tc.tile_pool(name="pep", bufs=4))
        cnp = ctx.enter_context(tc.tile_pool(name="cnp", bufs=4))
        osb = ctx.enter_context(tc.tile_pool(name="osb", bufs=4))

        x_tiles = {}

        def fetch_x(i):
            if i >= TI or i in x_tiles:
                return
            tiles = {}
            for t in TERMS:
                xt = xpool.tile([P, DC, P], FP8, tag=f"x{t}", name=f"xt_{t}{i}")
                nc.sync.dma_start(xt[:], xin[t][:, i])
                tiles[t] = xt
            x_tiles[i] = tiles

        def p1(i):
            """QKV projection + RMSNorm + RoPE + transposes for t-tile i."""
            xt = x_tiles.pop(i)
            fetch_x(i + 1)
            ps = {}
            for w in W_NAMES:
                pst = qkvps.tile([P, OC], F32, tag="qkv", name=f"ps_{w}{i}")
                first = True
                for t in TERMS:
                    for d in range(0, DC, 2):
                        nc.tensor.matmul(
                            pst[:],
                            lhsT=xt[t][:, d : d + 2, :],
                            rhs=w_sb[(w, t)][:, d : d + 2, :],
                            start=first,
                            stop=(t == TERMS[-1] and d == DC - 2),
                            perf_mode=DR,
                        )
                        first = False
                ps[w] = pst

            # V: combine + ones column already set
            nc.scalar.copy(
                v_sb[:, i, :, 0:HD],
                ps["v"][:].rearrange("p (h e) -> p h e", h=HPC),
            )

            cos3 = cos_sb[:, i : i + 1, :].to_broadcast((P, HPC, HD))
            sin_lo = sin_sb[:, i : i + 1, 0:64].to_broadcast((P, HPC, 64))
            sin_hi = sin_sb[:, i : i + 1, 64:HD].to_broadcast((P, HPC, 64))

            for w, rec, sqscale, sqbias in (
                ("q", recq, 1.0 / HD, eps_q),
                ("k", reck, 1.0, eps_k),
            ):
                q3 = ps[w][:].rearrange("p (h e) -> p h e", h=HPC)
                # RoPE directly from PSUM
                rA = work.tile([P, HPC, HD], BF16, tag=f"rA{w}")
                rB = work.tile([P, HPC, HD], BF16, tag=f"rB{w}")
                nc.vector.tensor_mul(rA[:], q3[:, :, :], cos3)
                nc.vector.tensor_mul(rB[:, :, 0:64], q3[:, :, 64:HD], sin_lo)
                nc.vector.tensor_mul(rB[:, :, 64:HD], q3[:, :, 0:64], sin_hi)
                qr = work.tile([P, HPC, HD], BF16, tag=f"qr{w}")
                nc.vector.tensor_add(qr[:], rA[:], rB[:])

                # RMS stats from the rotated vectors (rotation preserves norms)
                ssq = small.tile([P, HPC], F32, tag=f"ssq{w}")
                scr = work.tile([P, HD], BF16, tag="scr")
                for h in range(HPC):
                    nc.vector.scalar_tensor_tensor(
                        out=scr[:],
                        in0=qr[:, h, :],
                        scalar=1.0,
                        in1=qr[:, h, :],
                        op0=ALU.bypass,
                        op1=ALU.mult,
                        accum_out=ssq[:, h : h + 1],
                    )
                rms = small.tile([P, HPC], F32, tag=f"rms{w}")
                nc.scalar.activation(
                    rms[:], ssq[:], AF.Sqrt, bias=sqbias[:], scale=float(sqscale)
                )
                nc.vector.reciprocal(rec[:, i, :], rms[:])

                if w == "q":
                    # fold 1/rms into q per head (Act copy, per-partition scale)
                    qs = work.tile([P, HPC, HD], BF16, tag="qs")
                    for h in range(HPC):
                        nc.scalar.activation(
                            qs[:, h, :], qr[:, h, :], AF.Copy,
                            scale=recq[:, i, h : h + 1],
                        )
                    src, dst = qs, qT
                else:
                    src, dst = qr, kT
                for h in range(HPC):
                    nc.sync.dma_start_transpose(
                        dst[:, i, h, :], src[:, h, :]
                    )

        def p2(c, h):
            """Causal attention for head h, query chunk c (tq in [512c,512c+512))."""
            nca = 4 * c + 4
            cx = [
                cxps.tile([P, VW], F32, tag=f"cx{t}", name=f"cx{t}_{c}_{h}")[:]
                for t in range(4)
            ]
            for j in range(nca):
                off = max(0, j * P - c * 512)
                n = 512 - off
                t_lo = c * 512 + off
                i_lo = t_lo // P
                s_ps = sps.tile([P, 512], F32, tag="s")
                nc.tensor.matmul(
                    s_ps[:, 0:n],
                    lhsT=kT[:, j, h, :],
                    rhs=qT[:, i_lo : i_lo + (n // P), h, :],
                    start=True,
                    stop=True,
                )
                pe = pep.tile([P, 512], BF16, tag="pe")
                nc.scalar.activation(
                    pe[:, 0:n],
                    s_ps[:, 0:n],
                    AF.Exp,
                    scale=reck[:, j, h : h + 1],
                )
                if off > 0 or j * P == t_lo:
                    # diagonal block: first P columns need the causal mask
                    nc.vector.tensor_mul(pe[:, 0:P], pe[:, 0:P], mask_sb[:])
                for tsub in range(4):
                    i = 4 * c + tsub
                    if j > i:
                        continue
                    col0 = i * P - t_lo
                    nc.tensor.matmul(
                        cx[tsub],
                        lhsT=pe[:, col0 : col0 + P],
                        rhs=v_sb[:, j, h, :],
                        start=(j == 0),
                        stop=(j == i),
                    )
            for tsub in range(4):
                i = 4 * c + tsub
                rrs = small.tile([P, 1], F32, tag="rrs")
                nc.vector.reciprocal(rrs[:], cx[tsub][:, HD:VW])
                cn = cnp.tile([P, HD], BF16, tag="cn")
                nc.scalar.activation(
                    cn[:], cx[tsub][:, 0:HD], AF.Copy, scale=rrs[:]
                )
                nc.sync.dma_start_transpose(cT[:, i, h, :], cn[:])

        outv = out.rearrange("(ti tp) d -> tp ti d", tp=P)

        def p3(i):
            """Output projection for t-tile i."""
            for dc in range(4):
                po = qkvps.tile([P, OC], F32, tag="qkv", name=f"po{i}_{dc}")
                for h in range(HPC):
                    nc.tensor.matmul(
                        po[:],
                        lhsT=cT[:, i, h, :],
                        rhs=wot_sb[:, h, dc * 512 : (dc + 1) * 512],
                        start=(h == 0),
                        stop=(h == HPC - 1),
                    )
                ob = osb.tile([P, 512], BF16, tag="ob")
                if dc % 2 == 0:
                    nc.scalar.copy(ob[:], po[:])
                else:
                    nc.vector.tensor_copy(ob[:], po[:])
                nc.sync.dma_start(outv[:, i, dc * 512 : (dc + 1) * 512], ob[:])

        # ---- interleaved emission ----
        fetch_x(0)
        for i in range(4):
            p1(i)
        for c in range(4):
            for h in range(HPC):
                nxt = 4 * c + 4 + h
                if nxt < TI:
                    p1(nxt)
                p2(c, h)
            for i in range(4 * c, 4 * c + 4):
                p3(i)

        if dbg is not None:
            for nm, t in (("qT", qT), ("kT", kT), ("cT", cT), ("v", v_sb)):
                w = t.shape[-1]
                for i in range(TI):
                    cast = small.tile([P, HPC, VW], F32, tag="dbgcast")
                    nc.vector.tensor_copy(cast[:, :, 0:w], t[:, i])
                    nc.sync.dma_start(dbg[nm][:, i], cast[:, :, 0:w])
            nc.sync.dma_start(dbg["recq"], recq[:])
            nc.sync.dma_start(dbg["reck"], reck[:])


def _get_nc():
    if "nc" not in _NC_CACHE:
        _NC_CACHE["nc"] = _build_nc()
    return _NC_CACHE["nc"]


def _rope_tables():
    dim = HD // 2
    j = np.arange(dim, dtype=np.float64)
    freqs = np.exp(-j * np.log(ROPE_BASE) / dim)
    ang = np.arange(T, dtype=np.float64)[:, None] * freqs[None, :]
    cos = np.cos(ang)
    sin = np.sin(ang)
    cosf = np.concatenate([cos, cos], axis=1)   # [T, 128]
    sinf = np.concatenate([-sin, sin], axis=1)  # [T, 128], signed for the swap
    bf16 = ml_dtypes.bfloat16
    # [T, HD] -> [tp, ti, HD]
    cosf = cosf.reshape(TI, P, HD).transpose(1, 0, 2).astype(bf16).copy()
    sinf = sinf.reshape(TI, P, HD).transpose(1, 0, 2).astype(bf16).copy()
    return cosf, sinf


def _q8(a, scale=1.0):
    fp8 = ml_dtypes.float8_e4m3
    return np.clip(np.asarray(a, np.float32) * scale, -240, 240).astype(fp8)


def _prep_in_maps(x, Wq, Wk, Wv, Wo):
    bf16 = ml_dtypes.bfloat16
    perm = np.concatenate([np.arange(0, HD, 2), np.arange(1, HD, 2)])
    cosf, sinf = _rope_tables()
    maskd = np.triu(np.ones((P, P), dtype=np.float32)).astype(bf16)

    # x variants, [dp, ti, dc, tp]
    def xtile(a):
        return np.ascontiguousarray(
            a.reshape(TI, P, DC, P).transpose(3, 0, 2, 1)
        )

    xs = []
    for b in range(B):
        xb = x[b].astype(np.float32)
        x8 = _q8(xb)
        xl = _q8(xb - x8.astype(np.float32), RS)
        x8s = _q8(xb, 1.0 / RS)
        xs.append({"m": xtile(x8), "r": xtile(xl), "s": xtile(x8s)})

    in_maps = []
    for core in range(N_CORES):
        b, g = divmod(core, HPC)
        heads = g * HPC + np.arange(HPC)
        rows_perm = (heads[:, None] * HD + perm[None, :]).reshape(-1)
        rows_plain = (heads[:, None] * HD + np.arange(HD)[None, :]).reshape(-1)

        def wtiles(W, rows):
            # W[rows] is [OC, D]; -> [dp, dc, o] variants
            Wr = W[rows].astype(np.float32)
            W8 = _q8(Wr, SW)
            Ws = _q8(Wr, SW / RS)
            Wl = _q8(Wr * SW - W8.astype(np.float32), RS)
            def t(a):
                return np.ascontiguousarray(
                    a.T.reshape(DC, P, OC).transpose(1, 0, 2)
                )
            return {"m": t(W8), "r": t(Ws), "s": t(Wl)}

        wot_np = np.ascontiguousarray(
            Wo[:, rows_plain].T.reshape(HPC, HD, D).transpose(1, 0, 2)
        ).astype(bf16)
        m = {
            "wot": wot_np,
            "cosf": cosf,
            "sinf": sinf,
            "maskd": maskd,
        }
        for t in TERMS:
            m[f"x{t}"] = xs[b][t]
        for w, W, rows in (("q", Wq, rows_perm), ("k", Wk, rows_perm), ("v", Wv, rows_plain)):
            wt = wtiles(W, rows)
            for t in TERMS:
                m[f"w{t}{w}"] = wt[t]
        in_maps.append(m)
    return in_maps


def _numpy_reference(x, Wq, Wk, Wv, Wo, q_norm_w, k_norm_w):
    # exact fallback (only used if norm weights are not all-ones)
    q = (x.reshape(B * T, D) @ Wq.T).reshape(B, T, H, HD)
    k = (x.reshape(B * T, D) @ Wk.T).reshape(B, T, H, HD)
    v = (x.reshape(B * T, D) @ Wv.T).reshape(B, T, H, HD)

    def rms(t, w):
        n = np.sqrt(np.mean(np.square(t), axis=-1, keepdims=True) + EPS)
        return t / n * w

    q = rms(q, q_norm_w)
    k = rms(k, k_norm_w)
    dim = HD // 2
    freqs = np.exp(-np.arange(dim) * np.log(ROPE_BASE) / dim)
    ang = np.arange(T)[:, None] * freqs[None, :]
    cos = np.cos(ang)[None, :, None, :]
    sin = np.sin(ang)[None, :, None, :]

    def rope(t):
        e, o = t[..., ::2], t[..., 1::2]
        re = e * cos - o * sin
        ro = e * sin + o * cos
        return np.stack([re, ro], axis=-1).reshape(t.shape)

    q, k = rope(q), rope(k)
    scores = np.einsum("bthd,bshd->bhts", q, k) / np.sqrt(HD)
    causal = np.tril(np.ones((T, T), dtype=bool))
    scores = np.where(causal[None, None], scores, -1e30)
    scores -= scores.max(axis=-1, keepdims=True)
    p = np.exp(scores)
    p /= p.sum(axis=-1, keepdims=True)
    ctx = np.einsum("bhts,bshd->bthd", p, v).reshape(B, T, H * HD)
    return np.einsum("bto,do->btd", ctx, Wo).astype(np.float32)


def kernel(**inputs):
    x = np.asarray(inputs["x"], np.float32)
    Wq = np.asarray(inputs["Wq"], np.float32)
    Wk = np.asarray(inputs["Wk"], np.float32)
    Wv = np.asarray(inputs["Wv"], np.float32)
    Wo = np.asarray(inputs["Wo"], np.float32)
    qw = np.asarray(inputs["q_norm_w"], np.float32)
    kw = np.asarray(inputs["k_norm_w"], np.float32)

    if not (np.all(qw == 1.0) and np.all(kw == 1.0)):
        return _numpy_reference(x, Wq, Wk, Wv, Wo, qw, kw)

    out, _ = run(x, Wq, Wk, Wv, Wo)
    return out


def run(x, Wq, Wk, Wv, Wo, trace=False):
    nc = _get_nc()
    in_maps = _prep_in_maps(x, Wq, Wk, Wv, Wo)
    res = run_bass_kernel_spmd(
        nc, in_maps, core_ids=list(range(N_CORES)), trace=trace
    )
    parts = [r["out"].astype(np.float32) for r in res.results]
    out = np.stack(
        [
            parts[0] + parts[1] + parts[2] + parts[3],
            parts[4] + parts[5] + parts[6] + parts[7],
        ],
        axis=0,
    ) * (1.0 / SW)
    return out, res
